# revision 1
# baseline (speedup 1.0000x reference)
"""Multi-head attention (B=4, S=2048, D=1024, H=16, Dh=64) on 8 trn2 cores.

Sharding: core c -> batch b=c//2, head-group g=c%2 (8 heads = 512 qkv cols).
Host folds 1/sqrt(Dh) into Wq/bq, drops bk (softmax-invariant), splits bo
across the two cores of each batch. Each core computes a transposed partial
output outT [1024, 2048]; host sums core pairs and transposes.

Per-core dataflow:
  phase 1: qT/kT = W^T-chunks^T @ xT-chunks (transposed layout, d-on-partition),
           v natural (seq-on-partition) with a ones column per head (v_aug).
           x/W in bf16 (PE full rate); q bias folded into the PSUM eviction as
           a per-partition tensor_scalar add; v bias as a K=1 rank-1 matmul.
  phase 2: per head: S^T tiles = kT^T @ qT -> PSUM, exp on ACT -> P (SBUF bf16),
           ctx^T[65, q] += v_aug^T @ P (row 64 = softmax denominator l),
           normalize via reciprocal_approx_fast + partition_broadcast + DVE mult.
  phase 3: outT[e, q] = WoST^T @ ctxT_norm in float32r (output precision),
           bo/2 folded into eviction, DMA out.
"""
import numpy as np
import ml_dtypes
from contextlib import ExitStack

import concourse.bass as bass
import concourse.bacc as bacc
import concourse.mybir as mybir
import concourse.tile as tile
from concourse.bass_utils import run_bass_kernel_spmd

F32 = mybir.dt.float32
F32R = mybir.dt.float32r
BF16 = mybir.dt.bfloat16
NP_BF16 = ml_dtypes.bfloat16

B = 4
S = 2048
D = 1024
COLS = 512          # qkv cols per core (8 heads x 64)
NHEAD = 8           # heads per core
N = 512             # matmul moving free dim
DCH = D // 128      # 8 contraction chunks for projections
SC = S // N         # 4 seq chunks of 512
CC = COLS // 128    # 4 col chunks
KC = S // 128       # 16 key chunks
QH = S // 1024      # 2 query halves of 1024

_CACHE = {}


def _build():
    nc = bacc.Bacc("TRN2", target_bir_lowering=False, debug=False, num_devices=8)

    xt = nc.declare_dram_parameter("xt", [D, S], BF16, isOutput=False)
    wqt = nc.declare_dram_parameter("wqt", [D, COLS], BF16, isOutput=False)
    wkt = nc.declare_dram_parameter("wkt", [D, COLS], BF16, isOutput=False)
    wvt = nc.declare_dram_parameter("wvt", [D, COLS], BF16, isOutput=False)
    wot = nc.declare_dram_parameter("wot", [COLS, D], F32R, isOutput=False)
    bq = nc.declare_dram_parameter("bq", [128, CC], F32, isOutput=False)
    bv = nc.declare_dram_parameter("bv", [1, COLS], F32R, isOutput=False)
    bo2 = nc.declare_dram_parameter("bo2", [128, DCH], F32, isOutput=False)
    out = nc.declare_dram_parameter("out", [D, S], F32, isOutput=True)

    with ExitStack() as ctx:
        tc = ctx.enter_context(tile.TileContext(nc))

        const = ctx.enter_context(tc.tile_pool(name="const", bufs=1))
        ones_f32 = const.tile([128, 128], F32, tag="ones_f32")
        nc.vector.memset(ones_f32[:], 1.0)
        ones_r = const.tile([1, 128], F32R, tag="ones_r")
        nc.vector.tensor_copy(out=ones_r[:], in_=ones_f32[0:1, :])
        bq_t = const.tile([128, CC], F32, tag="bq")
        nc.sync.dma_start(out=bq_t[:], in_=bq[:])
        bv_t = const.tile([1, COLS], F32R, tag="bv")
        nc.sync.dma_start(out=bv_t[:], in_=bv[:])
        bo_t = const.tile([128, DCH], F32, tag="bo")
        nc.sync.dma_start(out=bo_t[:], in_=bo2[:])

        # persistent activations
        qkv = ctx.enter_context(tc.tile_pool(name="qkv", bufs=1))
        qT = [qkv.tile([128, S], BF16, tag=f"qt{c}", name=f"qt{c}") for c in range(CC)]
        kT = [qkv.tile([128, S], BF16, tag=f"kt{c}", name=f"kt{c}") for c in range(CC)]
        # v_aug: per seq chunk of 128 rows, 8 heads x (64 v cols + ones col)
        v_sb = [qkv.tile([128, NHEAD * 65], BF16, tag=f"v{i}", name=f"v{i}")
                for i in range(KC)]
        ctxn = ctx.enter_context(tc.tile_pool(name="ctxn", bufs=1))
        cn = [ctxn.tile([128, S], F32R, tag=f"cn{c}", name=f"cn{c}") for c in range(CC)]

        for i in range(KC):
            # ones column per head at local col 64
            va = v_sb[i][:].rearrange("p (h c) -> p h c", c=65)
            nc.vector.tensor_copy(
                out=va[:, :, 64:65],
                in_=ones_f32[:, 0:NHEAD].rearrange("p (h c) -> p h c", c=1),
            )

        # ---------------- phase 1: projections ----------------
        with tc.tile_pool(name="ph1ps", bufs=6, space="PSUM") as pp, \
             tc.tile_pool(name="xtp", bufs=16) as xtp, \
             tc.tile_pool(name="w", bufs=6) as wp, \
             tc.tile_pool(name="wv", bufs=4) as wvp:
            for sc in range(SC):
                xts = []
                for d in range(DCH):
                    t = xtp.tile([128, N], BF16, tag="xt", name=f"xt{sc}_{d}")
                    nc.sync.dma_start(
                        out=t[:], in_=xt[d * 128:(d + 1) * 128, sc * N:(sc + 1) * N]
                    )
                    xts.append(t)
                # qT, kT: [cols-chunk 128, seq 512] = sum_d W[d,c]^T @ xT[d,s]
                for proj, wsrc, bias in (("q", wqt, bq_t), ("k", wkt, None)):
                    dst = qT if proj == "q" else kT
                    for c in range(CC):
                        ps = pp.tile([128, N], F32, tag="ps", name=f"ps{proj}{sc}_{c}")
                        for d in range(DCH):
                            w_t = wp.tile([128, 128], BF16, tag="w",
                                          name=f"w{proj}{sc}_{c}_{d}")
                            nc.sync.dma_start(
                                out=w_t[:],
                                in_=wsrc[d * 128:(d + 1) * 128, c * 128:(c + 1) * 128],
                            )
                            nc.tensor.matmul(
                                ps[:], w_t[:], xts[d][:],
                                start=(d == 0), stop=(d == DCH - 1),
                            )
                        if bias is not None:
                            nc.vector.tensor_scalar_add(
                                out=dst[c][:, sc * N:(sc + 1) * N],
                                in0=ps[:],
                                scalar1=bias[:, c:c + 1],
                            )
                        else:
                            nc.vector.tensor_copy(
                                out=dst[c][:, sc * N:(sc + 1) * N], in_=ps[:]
                            )
                # v natural: [seq 128, cols 512] = sum_d xT[d, s128]^T @ WvT[d, :]
                vps = [pp.tile([128, N], F32, tag="ps", name=f"vps{sc}_{m}")
                       for m in range(4)]
                for d in range(DCH):
                    wv_t = wvp.tile([128, N], BF16, tag="wv", name=f"wv{sc}_{d}")
                    nc.sync.dma_start(
                        out=wv_t[:], in_=wvt[d * 128:(d + 1) * 128, :]
                    )
                    for m in range(4):
                        nc.tensor.matmul(
                            vps[m][:],
                            xts[d][:, m * 128:(m + 1) * 128],
                            wv_t[:],
                            start=(d == 0), stop=False,
                        )
                for m in range(4):
                    nc.tensor.matmul(
                        vps[m][:], ones_r[:], bv_t[:],
                        start=False, stop=True,
                    )
                    dst = v_sb[sc * 4 + m][:].rearrange("p (h c) -> p h c", c=65)
                    src = vps[m][:].rearrange("p (h c) -> p h c", c=64)
                    nc.vector.tensor_copy(out=dst[:, :, 0:64], in_=src[:])

        # ---------------- phase 2: attention per head ----------------
        with tc.tile_pool(name="stps", bufs=2, space="PSUM") as stp, \
             tc.tile_pool(name="ctxps", bufs=1, space="PSUM") as cxp, \
             tc.tile_pool(name="p", bufs=3) as pb, \
             tc.tile_pool(name="r", bufs=1) as rp, \
             tc.tile_pool(name="rb", bufs=1) as rbp:
            for h in range(NHEAD):
                c = h // 2
                po = (h % 2) * 64
                ctx_ps = cxp.tile([65, S], F32, tag="ctx", name=f"ctx{h}")
                for kc in range(KC):
                    lv = v_sb[kc][:, h * 65:(h + 1) * 65]
                    for qh in range(QH):
                        st = stp.tile([128, 1024], F32, tag="st", name=f"st{h}_{kc}_{qh}")
                        for qq in range(2):
                            qs = qh * 1024 + qq * N
                            nc.tensor.matmul(
                                st[:, qq * N:(qq + 1) * N],
                                kT[c][po:po + 64, kc * 128:(kc + 1) * 128],
                                qT[c][po:po + 64, qs:qs + N],
                                start=True, stop=True,
                            )
                        p_t = pb.tile([128, 1024], BF16, tag="p", name=f"p{h}_{kc}_{qh}")
                        nc.scalar.activation(
                            p_t[:], st[:], mybir.ActivationFunctionType.Exp
                        )
                        for qq in range(2):
                            qs = qh * 1024 + qq * N
                            nc.tensor.matmul(
                                ctx_ps[0:65, qs:qs + N],
                                lv,
                                p_t[:, qq * N:(qq + 1) * N],
                                start=(kc == 0), stop=(kc == KC - 1),
                            )
                # normalize: rows 0..63 /= row 64
                l_sb = rp.tile([1, S], F32, tag="l", name=f"l{h}")
                nc.vector.tensor_copy(out=l_sb[:], in_=ctx_ps[64:65, :])
                r_t = rp.tile([1, S], F32, tag="r", name=f"r{h}")
                nc.vector.reciprocal_approx_fast(r_t[:], l_sb[:])
                rb_t = rbp.tile([64, S], F32, tag="rb", name=f"rb{h}")
                nc.gpsimd.partition_broadcast(rb_t[:], r_t[:])
                nc.vector.tensor_tensor(
                    out=cn[c][po:po + 64, :],
                    in0=ctx_ps[0:64, :],
                    in1=rb_t[:],
                    op=mybir.AluOpType.mult,
                )

        # ---------------- phase 3: output projection ----------------
        with tc.tile_pool(name="outps", bufs=4, space="PSUM") as op, \
             tc.tile_pool(name="wo", bufs=6) as wop, \
             tc.tile_pool(name="outsb", bufs=4) as osb:
            for e in range(DCH):
                wo_ts = []
                for c2 in range(CC):
                    w_t = wop.tile([128, 128], F32R, tag="wo", name=f"wo{e}_{c2}")
                    nc.sync.dma_start(
                        out=w_t[:],
                        in_=wot[c2 * 128:(c2 + 1) * 128, e * 128:(e + 1) * 128],
                    )
                    wo_ts.append(w_t)
                for qc in range(SC):
                    ps = op.tile([128, N], F32, tag="ops", name=f"ops{e}_{qc}")
                    for c2 in range(CC):
                        nc.tensor.matmul(
                            ps[:], wo_ts[c2][:], cn[c2][:, qc * N:(qc + 1) * N],
                            start=(c2 == 0), stop=(c2 == CC - 1),
                        )
                    o_t = osb.tile([128, N], F32, tag="osb", name=f"osb{e}_{qc}")
                    nc.vector.tensor_scalar_add(
                        out=o_t[:], in0=ps[:], scalar1=bo_t[:, e:e + 1],
                    )
                    nc.sync.dma_start(
                        out=out[e * 128:(e + 1) * 128, qc * N:(qc + 1) * N],
                        in_=o_t[:],
                    )

    nc.compile()
    return nc


def _get_nc():
    if "nc" not in _CACHE:
        _CACHE["nc"] = _build()
    return _CACHE["nc"]


def _in_maps(x, Wq, bq, Wk, Wv, bv, Wo, bo):
    maps = []
    for core in range(8):
        b, g = core // 2, core % 2
        cols = slice(g * COLS, (g + 1) * COLS)
        maps.append({
            "xt": np.ascontiguousarray(x[b].T).astype(NP_BF16),
            "wqt": np.ascontiguousarray((Wq[cols] / 8.0).T).astype(NP_BF16),
            "bq": np.ascontiguousarray((bq[cols] / 8.0).reshape(CC, 128).T),
            "wkt": np.ascontiguousarray(Wk[cols].T).astype(NP_BF16),
            "wvt": np.ascontiguousarray(Wv[cols].T).astype(NP_BF16),
            "bv": bv[cols].reshape(1, COLS).copy(),
            "wot": np.ascontiguousarray(Wo[:, cols].T),
            "bo2": np.ascontiguousarray((bo / 2.0).reshape(DCH, 128).T),
        })
    return maps


def kernel(x, Wq, bq, Wk, bk, Wv, bv, Wo, bo, _trace=False, **trace_kwargs):
    x = np.asarray(x, dtype=np.float32)
    Wq = np.asarray(Wq, dtype=np.float32)
    bq = np.asarray(bq, dtype=np.float32)
    Wk = np.asarray(Wk, dtype=np.float32)
    Wv = np.asarray(Wv, dtype=np.float32)
    bv = np.asarray(bv, dtype=np.float32)
    Wo = np.asarray(Wo, dtype=np.float32)
    bo = np.asarray(bo, dtype=np.float32)

    nc = _get_nc()
    maps = _in_maps(x, Wq, bq, Wk, Wv, bv, Wo, bo)
    res = run_bass_kernel_spmd(nc, maps, list(range(8)), trace=_trace, **trace_kwargs)

    outp = np.empty((B, S, D), np.float32)
    for b in range(B):
        t = res.results[2 * b]["out"] + res.results[2 * b + 1]["out"]
        outp[b] = t.T
    if _trace:
        return outp, res
    return outp



# revision 19
# speedup vs baseline: 1.4265x; 1.4265x over previous
"""Multi-head attention (B=4, S=2048, D=1024, H=16, Dh=64) on 8 trn2 cores.

Sharding: core c -> batch b=c//2, head-group g=c%2 (8 heads = 512 qkv cols).
Host folds 1/sqrt(Dh) into Wq/bq, drops bk (softmax-invariant), splits bo
across the two cores of each batch. Each core computes a transposed partial
output outT [1024, 2048]; host sums core pairs and transposes.

v2: single software-pipelined stream built around the Scalar engine (exp is
the kernel's hard floor: 33.5M exps/core ~ 280us). Per (c=head-pair, qh=query
half) the kc loop computes row-packed score matmuls for heads A/B (K=64,
tile_position (0,0)/(64,0) concurrent), exps A then B ([128,1024] calls,
A/B alternation double-buffers the two st PSUM slots), and accumulates ctx
in one shared [65,1024] PSUM shuttle (4-kc groups: A prompt, B deferred
burst, evicted/accumulated to SBUF by DVE). The 2 remaining PSUM banks run
q/k/v projections and the output projection as "filler" groups drained at
<=1 per kc iteration so the PE never head-of-line-blocks the exp stream.
Weights are SBUF-resident; x streams per-c in a 16-tile ring.
"""
import numpy as np
import ml_dtypes
from collections import deque
from contextlib import ExitStack

import concourse.bass as bass
import concourse.bacc as bacc
import concourse.mybir as mybir
import concourse.tile as tile
from concourse.bass_utils import run_bass_kernel_spmd

F32 = mybir.dt.float32
F32R = mybir.dt.float32r
BF16 = mybir.dt.bfloat16
NP_BF16 = ml_dtypes.bfloat16

B = 4
S = 2048
D = 1024
COLS = 512          # qkv cols per core (8 heads x 64)
NHEAD = 8           # heads per core
N = 512             # matmul moving free dim
DCH = D // 128      # 8 contraction chunks for projections
SC = S // N         # 4 seq chunks of 512
CC = COLS // 128    # 4 col chunks (head pairs)
KC = S // 128       # 16 key chunks
QH = 2              # query halves of 1024
QHW = S // QH       # 1024

_CACHE = {}
_DEBUG = False


def _build():
    nc = bacc.Bacc("TRN2", target_bir_lowering=False, debug=False, num_devices=8)

    xt = nc.declare_dram_parameter("xt", [D, S], BF16, isOutput=False)
    wqt = nc.declare_dram_parameter("wqt", [D, COLS], BF16, isOutput=False)
    wkt = nc.declare_dram_parameter("wkt", [D, COLS], BF16, isOutput=False)
    wvt = nc.declare_dram_parameter("wvt", [D, COLS], BF16, isOutput=False)
    wot = nc.declare_dram_parameter("wot", [COLS, D], F32R, isOutput=False)
    bq = nc.declare_dram_parameter("bq", [128, CC], F32, isOutput=False)
    bv = nc.declare_dram_parameter("bv", [1, COLS], F32R, isOutput=False)
    bo2 = nc.declare_dram_parameter("bo2", [128, DCH], F32, isOutput=False)
    out = nc.declare_dram_parameter("out", [D, S], F32, isOutput=True)
    if _DEBUG:
        dbg_cn = nc.declare_dram_parameter("dbg_cn", [COLS, S], F32, isOutput=True)
        dbg_qt = nc.declare_dram_parameter("dbg_qt", [COLS, S], F32, isOutput=True)
        dbg_kt = nc.declare_dram_parameter("dbg_kt", [COLS, S], F32, isOutput=True)
        dbg_v = nc.declare_dram_parameter("dbg_v", [128, KC * NHEAD * 65], F32,
                                          isOutput=True)

    with ExitStack() as ctx:
        tc = ctx.enter_context(tile.TileContext(nc))

        # ---------------- persistent SBUF ----------------
        const = ctx.enter_context(tc.tile_pool(name="const", bufs=1))
        ones_f32 = const.tile([128, 128], F32, tag="ones_f32")
        nc.vector.memset(ones_f32[:], 1.0)
        bq_t = const.tile([128, CC], F32, tag="bq")
        nc.sync.dma_start(out=bq_t[:], in_=bq[:])
        ones_r = const.tile([1, 128], F32R, tag="ones_r")
        nc.vector.tensor_copy(out=ones_r[:], in_=ones_f32[0:1, :])
        bv_t = const.tile([1, COLS], F32R, tag="bv_t")
        nc.sync.dma_start(out=bv_t[:], in_=bv[:])
        bo_t = const.tile([128, DCH], F32, tag="bo")
        nc.sync.dma_start(out=bo_t[:], in_=bo2[:])

        # weights resident
        wpool = ctx.enter_context(tc.tile_pool(name="w", bufs=1))
        wq_sb = [wpool.tile([128, COLS], BF16, tag=f"wq{d}", name=f"wq{d}")
                 for d in range(DCH)]
        wk_sb = [wpool.tile([128, COLS], BF16, tag=f"wk{d}", name=f"wk{d}")
                 for d in range(DCH)]
        wv_sb = [wpool.tile([128, COLS], BF16, tag=f"wv{d}", name=f"wv{d}")
                 for d in range(DCH)]
        wo_sb = [wpool.tile([128, D], F32R, tag=f"wo{c}", name=f"wo{c}")
                 for c in range(CC)]
        for d in range(DCH):
            nc.sync.dma_start(out=wq_sb[d][:], in_=wqt[d * 128:(d + 1) * 128, :])
            nc.sync.dma_start(out=wk_sb[d][:], in_=wkt[d * 128:(d + 1) * 128, :])
            nc.sync.dma_start(out=wv_sb[d][:], in_=wvt[d * 128:(d + 1) * 128, :])
        for c in range(CC):
            nc.sync.dma_start(out=wo_sb[c][:], in_=wot[c * 128:(c + 1) * 128, :])

        # activations
        qkv = ctx.enter_context(tc.tile_pool(name="qkv", bufs=1))
        qT = [qkv.tile([128, S], BF16, tag=f"qt{c}", name=f"qt{c}") for c in range(CC)]
        kT = [qkv.tile([128, S], BF16, tag=f"kt{c}", name=f"kt{c}") for c in range(CC)]
        v_sb = [qkv.tile([128, NHEAD * 65], BF16, tag=f"v{i}", name=f"v{i}")
                for i in range(KC)]
        cn = [qkv.tile([128, S], F32R, tag=f"cn{c}", name=f"cn{c}") for c in range(CC)]

        for i in range(KC):
            va = v_sb[i][:].rearrange("p (h c) -> p h c", c=65)
            nc.vector.tensor_copy(
                out=va[:, :, 64:65],
                in_=ones_f32[:, 0:NHEAD].rearrange("p (h c) -> p h c", c=1),
            )

        # streaming x ring: 16 tile tags of [128, 512] (2 sc worth)
        xpool = ctx.enter_context(tc.tile_pool(name="x", bufs=1))
        # P tiles
        ppool = ctx.enter_context(tc.tile_pool(name="p", bufs=1))
        # ctx SBUF accumulators (per head, rotate across (c,qh))
        cxs = ctx.enter_context(tc.tile_pool(name="cxs", bufs=2))
        # normalize scratch
        rpool = ctx.enter_context(tc.tile_pool(name="r", bufs=1))
        rbpool = ctx.enter_context(tc.tile_pool(name="rb", bufs=1))
        # out staging
        opool = ctx.enter_context(tc.tile_pool(name="osb", bufs=3))

        # PSUM: stA(2) stB(2) shuttle(2) proj(2) = 8 banks
        psum = ctx.enter_context(tc.tile_pool(name="ps", bufs=1, space="PSUM"))

        # ---------------- filler machinery ----------------
        # fillers: deque of (key, fn). key None = DMA-only (free to pop).
        # Emission order of the queue is a valid topological order; drains
        # pop from the front so every filler's own prerequisites (earlier in
        # the queue) are emitted first.
        x_tiles = {}          # sc -> list of tiles (current ring contents)
        fillers = deque()
        done = set()

        def dma_x(sc):
            def go():
                tiles = []
                for d in range(DCH):
                    t = xpool.tile([128, N], BF16, tag=f"x{(sc % 2) * 8 + d}",
                                   name=f"x{sc}_{d}")
                    nc.sync.dma_start(
                        out=t[:], in_=xt[d * 128:(d + 1) * 128, sc * N:(sc + 1) * N]
                    )
                    tiles.append(t)
                x_tiles[sc] = tiles
            return go

        def proj_qk(proj, c, sc):
            def go():
                wsrc = wq_sb if proj == "q" else wk_sb
                dst = qT if proj == "q" else kT
                ps = psum.tile([128, N], F32, tag="proj", bufs=2,
                               name=f"ps{proj}{c}_{sc}")
                xs = x_tiles[sc]
                for d in range(DCH):
                    nc.tensor.matmul(
                        ps[:], wsrc[d][:, c * 128:(c + 1) * 128], xs[d][:],
                        start=(d == 0), stop=(d == DCH - 1),
                    )
                if proj == "q":
                    nc.vector.tensor_scalar_add(
                        out=dst[c][:, sc * N:(sc + 1) * N], in0=ps[:],
                        scalar1=bq_t[:, c:c + 1],
                    )
                else:
                    nc.vector.tensor_copy(
                        out=dst[c][:, sc * N:(sc + 1) * N], in_=ps[:]
                    )
            return go

        def proj_v(kc):
            def go():
                sc, m = kc // 4, kc % 4
                ps = psum.tile([128, COLS], F32, tag="proj", bufs=2,
                               name=f"psv{kc}")
                xs = x_tiles[sc]
                for d in range(DCH):
                    nc.tensor.matmul(
                        ps[:], xs[d][:, m * 128:(m + 1) * 128], wv_sb[d][:],
                        start=(d == 0), stop=False,
                    )
                nc.tensor.matmul(
                    ps[:], ones_r[:], bv_t[:], start=False, stop=True,
                )
                dst = v_sb[kc][:].rearrange("p (h c) -> p h c", c=65)
                src = ps[:].rearrange("p (h c) -> p h c", c=64)
                nc.vector.tensor_copy(out=dst[:, :, 0:64], in_=src[:])
            return go

        def out_proj(e, qc):
            def go():
                ps = psum.tile([128, N], F32, tag="proj", bufs=2,
                               name=f"pso{e}_{qc}")
                for c2 in range(CC):
                    nc.tensor.matmul(
                        ps[:], wo_sb[c2][:, e * 128:(e + 1) * 128],
                        cn[c2][:, qc * N:(qc + 1) * N],
                        start=(c2 == 0), stop=(c2 == CC - 1),
                    )
                o_t = opool.tile([128, N], F32, tag="o", name=f"o{e}_{qc}")
                nc.vector.tensor_scalar_add(
                    out=o_t[:], in0=ps[:], scalar1=bo_t[:, e:e + 1],
                )
                nc.sync.dma_start(
                    out=out[e * 128:(e + 1) * 128, qc * N:(qc + 1) * N],
                    in_=o_t[:],
                )
            return go

        def pop_one():
            key, fn = fillers.popleft()
            fn()
            if key is not None:
                done.add(key)
            return key

        def drain(n):
            """Emit up to n PE-work fillers (DMA-only pops are free)."""
            emitted = 0
            while fillers and emitted < n:
                if pop_one() is not None:
                    emitted += 1

        def drain_until(key):
            while key not in done:
                assert fillers, f"filler queue empty but {key} needed"
                pop_one()

        # filler order: c0 pass (with v), then c1..c3 passes
        fillers.append((None, dma_x(0)))
        fillers.append((None, dma_x(1)))
        fillers.append((("k", 0, 0), proj_qk("k", 0, 0)))
        fillers.append((("q", 0, 0), proj_qk("q", 0, 0)))
        fillers.append((("q", 0, 1), proj_qk("q", 0, 1)))
        for kc in range(4):
            fillers.append((("v", kc), proj_v(kc)))
        fillers.append((("k", 0, 1), proj_qk("k", 0, 1)))
        for kc in range(4, 8):
            fillers.append((("v", kc), proj_v(kc)))
        fillers.append((None, dma_x(2)))
        fillers.append((("k", 0, 2), proj_qk("k", 0, 2)))
        for kc in range(8, 12):
            fillers.append((("v", kc), proj_v(kc)))
        fillers.append((None, dma_x(3)))
        fillers.append((("k", 0, 3), proj_qk("k", 0, 3)))
        for kc in range(12, 16):
            fillers.append((("v", kc), proj_v(kc)))
        fillers.append((("q", 0, 2), proj_qk("q", 0, 2)))
        fillers.append((("q", 0, 3), proj_qk("q", 0, 3)))
        for c in range(1, CC):
            for sc in range(SC):
                if sc % 2 == 0:
                    fillers.append((None, dma_x(sc)))
                    fillers.append((None, dma_x(sc + 1)))
                fillers.append((("k", c, sc), proj_qk("k", c, sc)))
                fillers.append((("q", c, sc), proj_qk("q", c, sc)))

        # ---------------- main attention pipeline ----------------
        GK = 4  # kc per ctx shuttle group

        for qh in range(QH):
            if qh == 1:
                # queue output projection for query half 0
                for e in range(DCH):
                    for qc in range(2):
                        fillers.append((("o", e, qc), out_proj(e, qc)))
            for c in range(CC):
                drain_until(("q", c, 2 * qh))
                drain_until(("q", c, 2 * qh + 1))
                # ctx accumulators: out-of-place ping-pong across groups
                cacc = {
                    "A": [cxs.tile([65, QHW], F32, tag="cxA0", bufs=1,
                                   name=f"cxA0_{qh}_{c}"),
                          cxs.tile([65, QHW], F32, tag="cxA1", bufs=1,
                                   name=f"cxA1_{qh}_{c}")],
                    "B": [cxs.tile([65, QHW], F32, tag="cxB0", bufs=1,
                                   name=f"cxB0_{qh}_{c}"),
                          cxs.tile([65, QHW], F32, tag="cxB1", bufs=1,
                                   name=f"cxB1_{qh}_{c}")],
                }
                p_A = {}
                p_B = {}
                shuttle = None

                def evict_group(which, g, sh):
                    acc = cacc[which]
                    if g == 0:
                        nc.vector.tensor_copy(out=acc[0][:], in_=sh[:])
                    else:
                        src, dst = acc[(g + 1) % 2], acc[g % 2]
                        nc.vector.tensor_tensor(
                            out=dst[:], in0=sh[:], in1=src[:],
                            op=mybir.AluOpType.add,
                        )
                for kc in range(KC):
                    g, gi = kc // GK, kc % GK
                    drain_until(("k", c, kc // 4))
                    drain_until(("v", kc))
                    # scores: heads A (rows 0:64) and B (rows 64:128),
                    # adjacent emission -> concurrent row tiles
                    stA = psum.tile([128, QHW], F32, tag="stA",
                                    name=f"stA{qh}_{c}_{kc}")
                    stB = psum.tile([128, QHW], F32, tag="stB",
                                    name=f"stB{qh}_{c}_{kc}")
                    for po, st in ((0, stA), (64, stB)):
                        for qq in range(2):
                            qs = qh * QHW + qq * N
                            nc.tensor.matmul(
                                st[:, qq * N:(qq + 1) * N],
                                kT[c][po:po + 64, kc * 128:(kc + 1) * 128],
                                qT[c][po:po + 64, qs:qs + N],
                                start=True, stop=True,
                            )
                    pA = ppool.tile([128, QHW], BF16, tag="pA", bufs=4,
                                    name=f"pA{qh}_{c}_{kc}")
                    pB = ppool.tile([128, QHW], BF16, tag="pB", bufs=6,
                                    name=f"pB{qh}_{c}_{kc}")
                    nc.scalar.activation(
                        pA[:], stA[:], mybir.ActivationFunctionType.Exp
                    )
                    nc.scalar.activation(
                        pB[:], stB[:], mybir.ActivationFunctionType.Exp
                    )
                    p_A[kc] = pA
                    p_B[kc] = pB

                    # ctx: A prompt into shuttle
                    if gi == 0:
                        shuttle = psum.tile([65, QHW], F32, tag="shut",
                                            name=f"sh{qh}_{c}_{g}")
                    lvA = v_sb[kc][:, 2 * c * 65:(2 * c + 1) * 65]
                    for qq in range(2):
                        nc.tensor.matmul(
                            shuttle[:, qq * N:(qq + 1) * N],
                            lvA, p_A[kc][:, qq * N:(qq + 1) * N],
                            start=(gi == 0), stop=(gi == GK - 1),
                        )
                    drain(1)

                    if gi == GK - 1:
                        # evict A group, burst B group, evict B group
                        evict_group("A", g, shuttle)
                        shuttle = psum.tile([65, QHW], F32, tag="shut",
                                            name=f"shB{qh}_{c}_{g}")
                        for kb in range(g * GK, (g + 1) * GK):
                            lvB = v_sb[kb][:, (2 * c + 1) * 65:(2 * c + 2) * 65]
                            for qq in range(2):
                                nc.tensor.matmul(
                                    shuttle[:, qq * N:(qq + 1) * N],
                                    lvB, p_B[kb][:, qq * N:(qq + 1) * N],
                                    start=(kb == g * GK), stop=(kb == (g + 1) * GK - 1),
                                )
                        evict_group("B", g, shuttle)
                        shuttle = None

                # normalize: rows 0..63 /= row 64; write cn
                last = (KC // GK - 1) % 2
                for name, po in (("A", 0), ("B", 64)):
                    cx = cacc[name][last]
                    l_t = rpool.tile([1, QHW], F32, tag=f"l{name}",
                                     name=f"l{name}{qh}_{c}")
                    nc.vector.tensor_copy(out=l_t[:], in_=cx[64:65, :])
                    r_t = rpool.tile([1, QHW], F32, tag=f"r{name}",
                                     name=f"r{name}{qh}_{c}")
                    nc.vector.reciprocal_approx_fast(r_t[:], l_t[:])
                    rb_t = rbpool.tile([64, QHW], F32, tag=f"rb{name}",
                                       name=f"rb{name}{qh}_{c}")
                    nc.gpsimd.partition_broadcast(rb_t[:], r_t[:])
                    nc.vector.tensor_tensor(
                        out=cn[c][po:po + 64, qh * QHW:(qh + 1) * QHW],
                        in0=cx[0:64, :], in1=rb_t[:],
                        op=mybir.AluOpType.mult,
                    )

        # tail: remaining fillers + output projection for query half 1
        for e in range(DCH):
            for qc in range(2, SC):
                fillers.append((("o", e, qc), out_proj(e, qc)))
        while fillers:
            pop_one()

        if _DEBUG:
            dbgp = ctx.enter_context(tc.tile_pool(name="dbg", bufs=1))
            for c in range(CC):
                for blk, dram in ((cn[c], dbg_cn), (qT[c], dbg_qt), (kT[c], dbg_kt)):
                    for hh in range(2):
                        t = dbgp.tile([128, S // 2], F32, tag="d", name=f"dbg{c}")
                        nc.vector.tensor_copy(
                            out=t[:], in_=blk[:, hh * (S // 2):(hh + 1) * (S // 2)]
                        )
                        nc.sync.dma_start(
                            out=dram[c * 128:(c + 1) * 128,
                                     hh * (S // 2):(hh + 1) * (S // 2)],
                            in_=t[:],
                        )
            for i in range(KC):
                t = dbgp.tile([128, NHEAD * 65], F32, tag="dv", name=f"dbgv{i}")
                nc.vector.tensor_copy(out=t[:], in_=v_sb[i][:])
                nc.sync.dma_start(
                    out=dbg_v[:, i * NHEAD * 65:(i + 1) * NHEAD * 65], in_=t[:]
                )

    nc.compile()
    return nc


def _get_nc():
    if "nc" not in _CACHE:
        _CACHE["nc"] = _build()
    return _CACHE["nc"]


def _in_maps(x, Wq, bq, Wk, Wv, bv, Wo, bo):
    maps = []
    for core in range(8):
        b, g = core // 2, core % 2
        cols = slice(g * COLS, (g + 1) * COLS)
        maps.append({
            "xt": np.ascontiguousarray(x[b].T).astype(NP_BF16),
            "wqt": np.ascontiguousarray((Wq[cols] / 8.0).T).astype(NP_BF16),
            "bq": np.ascontiguousarray((bq[cols] / 8.0).reshape(CC, 128).T),
            "wkt": np.ascontiguousarray(Wk[cols].T).astype(NP_BF16),
            "wvt": np.ascontiguousarray(Wv[cols].T).astype(NP_BF16),
            "bv": bv[cols].reshape(1, COLS).copy(),
            "wot": np.ascontiguousarray(Wo[:, cols].T),
            "bo2": np.ascontiguousarray((bo / 2.0).reshape(DCH, 128).T),
        })
    return maps


def kernel(x, Wq, bq, Wk, bk, Wv, bv, Wo, bo, _trace=False, **trace_kwargs):
    x = np.asarray(x, dtype=np.float32)
    Wq = np.asarray(Wq, dtype=np.float32)
    bq = np.asarray(bq, dtype=np.float32)
    Wk = np.asarray(Wk, dtype=np.float32)
    Wv = np.asarray(Wv, dtype=np.float32)
    bv = np.asarray(bv, dtype=np.float32)
    Wo = np.asarray(Wo, dtype=np.float32)
    bo = np.asarray(bo, dtype=np.float32)

    nc = _get_nc()
    maps = _in_maps(x, Wq, bq, Wk, Wv, bv, Wo, bo)
    res = run_bass_kernel_spmd(nc, maps, list(range(8)), trace=_trace, **trace_kwargs)

    outp = np.empty((B, S, D), np.float32)
    for b in range(B):
        t = res.results[2 * b]["out"] + res.results[2 * b + 1]["out"]
        outp[b] = t.T
    if _trace:
        return outp, res
    return outp
